# Initial kernel scaffold
#
"""Channel-attention kernel for Trainium2 (8 NeuronCores, SPMD data-parallel).

Computes, per sample b:
    xv = x[b].reshape(C, N)
    G  = xv @ xv.T              (C x C gram, symmetric)
    S  = softmax(G, axis=-1)
    v  = S @ xv
    out[b] = alpha * v + x[b]

Sharding: batch (B=32) split 4-per-core across 8 cores. No collectives.

Implementation notes:
 - Matmuls run in bf16 (1 cycle/row on the PE vs 4 for fp32). PSUM accumulates
   fp32. The final output is alpha * value + x with x added in exact fp32.
 - The gram matrix is symmetric, so the SBUF tiles holding G (partition=c,
   free=d) are reinterpreted as the transposed view (partition=d, free=c)
   needed as the stationary operand of the second matmul. No transpose of the
   1024x1024 matrix is ever done.
 - X^T (needed for the gram matmul: contraction over spatial N must be on
   partitions) is produced with the DMA xbar transpose (bf16). The spatial
   rows land in a permuted order, which is harmless: the contraction over N is
   order-invariant and both matmul operands use the same buffer.
 - A ones-column is appended to the bf16 copy of x; in the second matmul it
   yields the softmax denominators (row sums of exp) directly in PSUM. Its
   transposed counterpart (a ones-row in X^T) only adds a constant +1 to every
   gram entry, which softmax cancels exactly.
 - Row maxes are exact (DVE reduce over the free axis of the view-1 tiles =
   row max by symmetry); broadcast along partitions via a PE transpose and two
   K=1 bf16 matmuls (a bf16-rounded max only shifts exp args by a per-row
   constant, which softmax cancels).
 - Software pipeline: bmm1 of sample s+1 is emitted between softmax(s) and
   bmm2(s), so the PE never waits for the softmax chain in steady state.
"""

import numpy as np

B, C, H, W = 32, 1024, 28, 28
N = H * W            # 784
NCORES = 8
SPC = B // NCORES    # samples per core
NPAD = 896           # next multiple of 128 >= N+1 (ones col at index N)
P = 128


def build_nc(spc=SPC, c=C, n=N, npad=NPAD):
    from contextlib import ExitStack

    import concourse.bass as bass
    import concourse.tile as tile
    from concourse import bacc, mybir
    from concourse.masks import make_identity

    FP = mybir.dt.float32
    BF = mybir.dt.bfloat16
    AX = mybir.AxisListType
    ALU = mybir.AluOpType
    ACTF = mybir.ActivationFunctionType

    mt = c // P          # m-tiles over channels (also d-tiles)
    kt = npad // P       # k-tiles for the gram matmul
    nf = n + 1           # free width of second matmul (value cols + sum col)

    nc = bacc.Bacc("TRN2", target_bir_lowering=False, debug=False)
    x_d = nc.declare_dram_parameter("x", [spc, c, n], FP, isOutput=False)
    a_d = nc.declare_dram_parameter("alpha", [1, 1], FP, isOutput=False)
    o_d = nc.declare_dram_parameter("out", [spc, c, n], FP, isOutput=True)

    with tile.TileContext(nc) as tc, ExitStack() as ctx:
        singles = ctx.enter_context(tc.tile_pool(name="singles", bufs=1))
        xbf_p = ctx.enter_context(tc.tile_pool(name="xbf", bufs=3))
        xt_p = ctx.enter_context(tc.tile_pool(name="xt", bufs=8))
        xnc_p = ctx.enter_context(tc.tile_pool(name="xnc", bufs=2))
        g_p = ctx.enter_context(tc.tile_pool(name="g", bufs=mt))
        e_p = ctx.enter_context(tc.tile_pool(name="e", bufs=2))
        mx_p = ctx.enter_context(tc.tile_pool(name="mx", bufs=2))
        xa_p = ctx.enter_context(tc.tile_pool(name="xa", bufs=3))
        sv_p = ctx.enter_context(tc.tile_pool(name="sv", bufs=4))
        ps_p = ctx.enter_context(tc.tile_pool(name="ps", bufs=2, space="PSUM"))
        psr_p = ctx.enter_context(tc.tile_pool(name="psr", bufs=1, space="PSUM"))
        psm_p = ctx.enter_context(tc.tile_pool(name="psm", bufs=1, space="PSUM"))

        identity = singles.tile([P, P], FP)
        make_identity(nc, identity)

        ones_bf = singles.tile([1, P], BF)
        nc.vector.memset(ones_bf, 1.0)

        # alpha -> per-partition column (128, 1)
        alpha_sb = singles.tile([1, 1], BF)
        nc.gpsimd.dma_start(out=alpha_sb, in_=a_d[:, :])
        alpha_ps = ps_p.tile([P, 1], FP, tag="ps")
        nc.tensor.matmul(alpha_ps, ones_bf, alpha_sb, start=True, stop=True)
        alpha_col = singles.tile([P, 1], FP)
        nc.vector.tensor_copy(alpha_col, alpha_ps)

        xbf_t = [None] * spc
        xnc_t = [None] * spc
        g_t = [[None] * mt for _ in range(spc)]
        e_t = [None] * spc
        mxb_t = [None] * spc

        def emit_prep(s):
            """DMA-cast x[s] to bf16 (+ones col, zero pad) and xbar-transpose."""
            xbf = xbf_p.tile([P, mt, npad], BF, tag="xbf")
            xbf_t[s] = xbf
            nc.gpsimd.memset(xbf[:, :, n : n + 1], 1.0)
            if npad > nf:
                nc.gpsimd.memset(xbf[:, :, nf:npad], 0.0)
            xnc = xnc_p.tile([P, kt, c], BF, tag="xnc")
            xnc_t[s] = xnc
            for t in range(mt):
                # HWDGE f32 load, then cast to bf16 split across DVE/ACT so
                # the casts don't all queue behind one engine's softmax work
                xt = xt_p.tile([P, n], FP, tag="xt")
                nc.sync.dma_start(out=xt, in_=x_d[s, P * t : P * (t + 1), :])
                if t % 2 == 0:
                    nc.vector.tensor_copy(xbf[:, t, 0:n], xt)
                else:
                    nc.scalar.copy(xbf[:, t, 0:n], xt)
                nc.sync.dma_start_transpose(
                    out=xnc[:, :, P * t : P * (t + 1)], in_=xbf[:, t, :]
                )

        def emit_bmm1(s):
            """Gram matmul, per-row maxes, and their partition-broadcast."""
            xnc = xnc_t[s]
            mx8 = mx_p.tile([P, mt], FP, tag="mx8")
            psr = psr_p.tile([1, c], FP, tag="psr")
            for m in range(mt):
                ps = ps_p.tile([P, c], FP, tag="ps")
                for k in range(kt):
                    lhsT = xnc[:, k, P * m : P * (m + 1)]
                    st, sp = (k == 0), (k == kt - 1)
                    for h in range(0, c, 512):
                        hw_ = min(512, c - h)
                        nc.tensor.matmul(
                            ps[:, h : h + hw_],
                            lhsT,
                            xnc[:, k, h : h + hw_],
                            start=st,
                            stop=sp,
                        )
                g = g_p.tile([P, c], FP, tag="g")
                g_t[s][m] = g
                nc.vector.reduce_max(out=mx8[:, m : m + 1], in_=ps, axis=AX.X)
                nc.any.tensor_copy(g, ps)
                nc.tensor.transpose(
                    psr[0:1, P * m : P * (m + 1)], mx8[:, m : m + 1], identity
                )
            # broadcast row maxes along partitions (K=1 matmuls, bf16)
            mx_row = mx_p.tile([1, c], BF, tag="mxrow")
            nc.vector.tensor_copy(mx_row, psr)
            mxb = psm_p.tile([P, c], FP, tag="psm")
            mxb_t[s] = mxb
            for h in range(0, c, 512):
                hw_ = min(512, c - h)
                nc.tensor.matmul(
                    mxb[:, h : h + hw_],
                    ones_bf,
                    mx_row[0:1, h : h + hw_],
                    start=True,
                    stop=True,
                )

        def emit_softmax(s):
            """exp(G - rowmax) written transposed-by-symmetry, bf16."""
            e = e_p.tile([P, mt, c], BF, tag="e")
            e_t[s] = e
            mxb = mxb_t[s]
            for t in range(mt):
                g = g_t[s][t]
                nc.vector.tensor_sub(g, g, mxb)
                nc.scalar.activation(e[:, t, :], g, ACTF.Exp)

        def emit_bmm2(s):
            """value = E^T @ X (+ sum col), normalize, add x, store."""
            e = e_t[s]
            xbf = xbf_t[s]
            for m in range(mt):
                xat = xa_p.tile([P, n], FP, tag="xa")
                nc.sync.dma_start(out=xat, in_=x_d[s, P * m : P * (m + 1), :])
                ps2 = ps_p.tile([P, nf], FP, tag="ps")
                for k in range(mt):
                    lhsT = e[:, k, P * m : P * (m + 1)]
                    st, sp = (k == 0), (k == mt - 1)
                    for h in range(0, nf, 512):
                        hw_ = min(512, nf - h)
                        nc.tensor.matmul(
                            ps2[:, h : h + hw_],
                            lhsT,
                            xbf[:, k, h : h + hw_],
                            start=st,
                            stop=sp,
                        )
                rec = sv_p.tile([P, 1], FP, tag="rec")
                nc.vector.reciprocal(rec, ps2[:, n : n + 1])
                scale = sv_p.tile([P, 1], FP, tag="scale")
                nc.vector.tensor_mul(scale, rec, alpha_col)
                nc.vector.scalar_tensor_tensor(
                    out=xat,
                    in0=ps2[:, 0:n],
                    scalar=scale,
                    in1=xat,
                    op0=ALU.mult,
                    op1=ALU.add,
                )
                nc.sync.dma_start(out=o_d[s, P * m : P * (m + 1), :], in_=xat)

        emit_prep(0)
        emit_bmm1(0)
        for s in range(spc):
            emit_softmax(s)
            if s + 1 < spc:
                emit_prep(s + 1)
                emit_bmm1(s + 1)
            emit_bmm2(s)

    nc.compile()
    return nc


def kernel(x, alpha):
    from concourse.bass_utils import run_bass_kernel_spmd

    x = np.ascontiguousarray(x, dtype=np.float32).reshape(B, C, N)
    alpha = np.asarray(alpha, dtype=np.float32).reshape(1, 1)
    nc = build_nc()
    in_maps = [
        {"x": np.ascontiguousarray(x[i * SPC : (i + 1) * SPC]), "alpha": alpha}
        for i in range(NCORES)
    ]
    res = run_bass_kernel_spmd(nc, in_maps, core_ids=list(range(NCORES)))
    out = np.concatenate([r["out"] for r in res.results], axis=0)
    return out.reshape(B, C, H, W)


if __name__ == "__main__":
    import reference

    inputs = reference.setup_inputs()
    expected = np.asarray(reference.reference(**inputs))
    actual = kernel(np.asarray(inputs["x"]), np.asarray(inputs["alpha"]))
    err = np.abs(actual - expected).max()
    rel = np.linalg.norm(actual - expected) / max(np.linalg.norm(expected), 1e-30)
    print("max abs err:", err, "rel err:", rel)



# revision 1
# speedup vs baseline: 66.6670x; 66.6670x over previous
"""Channel-attention kernel for Trainium2 (8 NeuronCores, SPMD data-parallel).

Computes, per sample b:
    xv = x[b].reshape(C, N)
    G  = xv @ xv.T              (C x C gram, symmetric)
    S  = softmax(G, axis=-1)
    v  = S @ xv
    out[b] = alpha * v + x[b]

Sharding: batch (B=32) split 4-per-core across 8 cores. No collectives.

Implementation notes:
 - Matmuls run in bf16 (1 cycle/row on the PE vs 4 for fp32). PSUM accumulates
   fp32. The final output is alpha * value + x with x added in exact fp32.
 - The gram matrix is symmetric, so the SBUF tiles holding G (partition=c,
   free=d) are reinterpreted as the transposed view (partition=d, free=c)
   needed as the stationary operand of the second matmul. No transpose of the
   1024x1024 matrix is ever done.
 - X^T (needed for the gram matmul: contraction over spatial N must be on
   partitions) is produced with the DMA xbar transpose (bf16). The spatial
   rows land in a permuted order, which is harmless: the contraction over N is
   order-invariant and both matmul operands use the same buffer.
 - A ones-column is appended to the bf16 copy of x; in the second matmul it
   yields the softmax denominators (row sums of exp) directly in PSUM. Its
   transposed counterpart (a ones-row in X^T) only adds a constant +1 to every
   gram entry, which softmax cancels exactly.
 - Row maxes are exact (DVE reduce over the free axis of the view-1 tiles =
   row max by symmetry); broadcast along partitions via a PE transpose and two
   K=1 bf16 matmuls (a bf16-rounded max only shifts exp args by a per-row
   constant, which softmax cancels).
 - Software pipeline: bmm1 of sample s+1 is emitted between softmax(s) and
   bmm2(s), so the PE never waits for the softmax chain in steady state.
"""

import numpy as np

B, C, H, W = 32, 1024, 28, 28
N = H * W            # 784
NCORES = 8
SPC = B // NCORES    # samples per core
NPAD = 896           # next multiple of 128 >= N+1 (ones col at index N)
P = 128


def build_nc(spc=SPC, c=C, n=N, npad=NPAD):
    from contextlib import ExitStack

    import concourse.bass as bass
    import concourse.tile as tile
    from concourse import bacc, mybir
    from concourse.masks import make_identity

    FP = mybir.dt.float32
    BF = mybir.dt.bfloat16
    AX = mybir.AxisListType
    ALU = mybir.AluOpType
    ACTF = mybir.ActivationFunctionType

    mt = c // P          # m-tiles over channels (also d-tiles)
    kt = npad // P       # k-tiles for the gram matmul
    nf = n + 1           # free width of second matmul (value cols + sum col)

    nc = bacc.Bacc("TRN2", target_bir_lowering=False, debug=False)
    x_d = nc.declare_dram_parameter("x", [spc, c, n], FP, isOutput=False)
    a_d = nc.declare_dram_parameter("alpha", [1, 1], FP, isOutput=False)
    o_d = nc.declare_dram_parameter("out", [spc, c, n], FP, isOutput=True)

    with tile.TileContext(nc) as tc, ExitStack() as ctx:
        singles = ctx.enter_context(tc.tile_pool(name="singles", bufs=1))
        xbf_p = ctx.enter_context(tc.tile_pool(name="xbf", bufs=3))
        xt_p = ctx.enter_context(tc.tile_pool(name="xt", bufs=8))
        xnc_p = ctx.enter_context(tc.tile_pool(name="xnc", bufs=2))
        g_p = ctx.enter_context(tc.tile_pool(name="g", bufs=mt))
        e_p = ctx.enter_context(tc.tile_pool(name="e", bufs=2))
        mx_p = ctx.enter_context(tc.tile_pool(name="mx", bufs=2))
        xa_p = ctx.enter_context(tc.tile_pool(name="xa", bufs=3))
        sv_p = ctx.enter_context(tc.tile_pool(name="sv", bufs=4))
        ps_p = ctx.enter_context(tc.tile_pool(name="ps", bufs=2, space="PSUM"))
        psr_p = ctx.enter_context(tc.tile_pool(name="psr", bufs=1, space="PSUM"))
        psm_p = ctx.enter_context(tc.tile_pool(name="psm", bufs=1, space="PSUM"))

        identity = singles.tile([P, P], FP)
        make_identity(nc, identity)

        ones_bf = singles.tile([1, P], BF)
        nc.vector.memset(ones_bf, 1.0)

        # alpha -> per-partition column (128, 1)
        alpha_sb = singles.tile([1, 1], BF)
        nc.gpsimd.dma_start(out=alpha_sb, in_=a_d[:, :])
        alpha_ps = ps_p.tile([P, 1], FP, tag="ps")
        nc.tensor.matmul(alpha_ps, ones_bf, alpha_sb, start=True, stop=True)
        alpha_col = singles.tile([P, 1], FP)
        nc.vector.tensor_copy(alpha_col, alpha_ps)

        xbf_t = [None] * spc
        xnc_t = [None] * spc
        g_t = [[None] * mt for _ in range(spc)]
        e_t = [None] * spc
        mxb_t = [None] * spc

        def emit_prep(s):
            """DMA-cast x[s] to bf16 (+ones col, zero pad) and xbar-transpose."""
            xbf = xbf_p.tile([P, mt, npad], BF, tag="xbf")
            xbf_t[s] = xbf
            nc.gpsimd.memset(xbf[:, :, n : n + 1], 1.0)
            if npad > nf:
                nc.gpsimd.memset(xbf[:, :, nf:npad], 0.0)
            xnc = xnc_p.tile([P, kt, c], BF, tag="xnc")
            xnc_t[s] = xnc
            for t in range(mt):
                # HWDGE f32 load, then cast to bf16 split across DVE/ACT so
                # the casts don't all queue behind one engine's softmax work
                xt = xt_p.tile([P, n], FP, tag="xt")
                nc.sync.dma_start(out=xt, in_=x_d[s, P * t : P * (t + 1), :])
                if t % 2 == 0:
                    nc.vector.tensor_copy(xbf[:, t, 0:n], xt)
                else:
                    nc.scalar.copy(xbf[:, t, 0:n], xt)
                nc.sync.dma_start_transpose(
                    out=xnc[:, :, P * t : P * (t + 1)], in_=xbf[:, t, :]
                )

        def emit_bmm1(s):
            """Gram matmul, per-row maxes, and their partition-broadcast."""
            xnc = xnc_t[s]
            mx8 = mx_p.tile([P, mt], FP, tag="mx8")
            psr = psr_p.tile([1, c], FP, tag="psr")
            for m in range(mt):
                ps = ps_p.tile([P, c], FP, tag="ps")
                for k in range(kt):
                    lhsT = xnc[:, k, P * m : P * (m + 1)]
                    st, sp = (k == 0), (k == kt - 1)
                    for h in range(0, c, 512):
                        hw_ = min(512, c - h)
                        nc.tensor.matmul(
                            ps[:, h : h + hw_],
                            lhsT,
                            xnc[:, k, h : h + hw_],
                            start=st,
                            stop=sp,
                        )
                g = g_p.tile([P, c], FP, tag="g")
                g_t[s][m] = g
                nc.vector.reduce_max(out=mx8[:, m : m + 1], in_=ps, axis=AX.X)
                nc.any.tensor_copy(g, ps)
                nc.tensor.transpose(
                    psr[0:1, P * m : P * (m + 1)], mx8[:, m : m + 1], identity
                )
            # broadcast row maxes along partitions (K=1 matmuls, bf16)
            mx_row = mx_p.tile([1, c], BF, tag="mxrow")
            nc.vector.tensor_copy(mx_row, psr)
            mxb = psm_p.tile([P, c], FP, tag="psm")
            mxb_t[s] = mxb
            for h in range(0, c, 512):
                hw_ = min(512, c - h)
                nc.tensor.matmul(
                    mxb[:, h : h + hw_],
                    ones_bf,
                    mx_row[0:1, h : h + hw_],
                    start=True,
                    stop=True,
                )

        def emit_softmax(s):
            """exp(G - rowmax) written transposed-by-symmetry, bf16."""
            e = e_p.tile([P, mt, c], BF, tag="e")
            e_t[s] = e
            mxb = mxb_t[s]
            for t in range(mt):
                g = g_t[s][t]
                nc.vector.tensor_sub(g, g, mxb)
                nc.scalar.activation(e[:, t, :], g, ACTF.Exp)

        def emit_bmm2(s):
            """value = E^T @ X (+ sum col), normalize, add x, store."""
            e = e_t[s]
            xbf = xbf_t[s]
            for m in range(mt):
                xat = xa_p.tile([P, n], FP, tag="xa")
                nc.sync.dma_start(out=xat, in_=x_d[s, P * m : P * (m + 1), :])
                ps2 = ps_p.tile([P, nf], FP, tag="ps")
                for k in range(mt):
                    lhsT = e[:, k, P * m : P * (m + 1)]
                    st, sp = (k == 0), (k == mt - 1)
                    for h in range(0, nf, 512):
                        hw_ = min(512, nf - h)
                        nc.tensor.matmul(
                            ps2[:, h : h + hw_],
                            lhsT,
                            xbf[:, k, h : h + hw_],
                            start=st,
                            stop=sp,
                        )
                rec = sv_p.tile([P, 1], FP, tag="rec")
                nc.vector.reciprocal(rec, ps2[:, n : n + 1])
                scale = sv_p.tile([P, 1], FP, tag="scale")
                nc.vector.tensor_mul(scale, rec, alpha_col)
                nc.vector.scalar_tensor_tensor(
                    out=xat,
                    in0=ps2[:, 0:n],
                    scalar=scale,
                    in1=xat,
                    op0=ALU.mult,
                    op1=ALU.add,
                )
                nc.sync.dma_start(out=o_d[s, P * m : P * (m + 1), :], in_=xat)

        emit_prep(0)
        emit_bmm1(0)
        for s in range(spc):
            emit_softmax(s)
            if s + 1 < spc:
                emit_prep(s + 1)
                emit_bmm1(s + 1)
            emit_bmm2(s)

    nc.compile()
    return nc


def kernel(x, alpha):
    from concourse.bass_utils import run_bass_kernel_spmd

    x = np.ascontiguousarray(x, dtype=np.float32).reshape(B, C, N)
    alpha = np.asarray(alpha, dtype=np.float32).reshape(1, 1)
    nc = build_nc()
    in_maps = [
        {"x": np.ascontiguousarray(x[i * SPC : (i + 1) * SPC]), "alpha": alpha}
        for i in range(NCORES)
    ]
    res = run_bass_kernel_spmd(nc, in_maps, core_ids=list(range(NCORES)))
    out = np.concatenate([r["out"] for r in res.results], axis=0)
    return out.reshape(B, C, H, W)


if __name__ == "__main__":
    import reference

    inputs = reference.setup_inputs()
    expected = np.asarray(reference.reference(**inputs))
    actual = kernel(np.asarray(inputs["x"]), np.asarray(inputs["alpha"]))
    err = np.abs(actual - expected).max()
    rel = np.linalg.norm(actual - expected) / max(np.linalg.norm(expected), 1e-30)
    print("max abs err:", err, "rel err:", rel)

